# revision 1
# baseline (speedup 1.0000x reference)
"""GuidedCrossAttention Trainium2 kernel (v10, ~57us HW vs 185us fp32 baseline).

Sharding: 16 graphs -> 8 cores, 2 graphs per core. Graphs are paired
big-with-small by key count into two SLOTS with their own padded sizes
(QB0/KB0, QB1/KB1), so the pad waste of the uniform-max scheme is roughly
halved while every core still runs the identical program.

Per core: block-diagonal attention over its two graphs. All projections are
host-folded into single effective matrices (SCALE folded into Wq_eff; v-bias +
out-proj biases folded into the residual term added to xqtok on host).

Key device-side structure:
  - all matmuls bf16 (fp32 LOW_HIGH mode is ~4x slower per column)
  - heads at natural 32-row offsets; score matmuls use tile_position row
    groups, U/denominator matmuls use col groups (concurrent PE quadrants)
  - heads processed in PAIRS with two 2-bank S PSUM tiles ping-ponging so the
    scalar engine's exp activates run back-to-back instead of WAR-stalling
  - denominator: pad keys killed by -30000 exp bias (mask column); d computed
    by an M=32 all-ones matmul so every row holds d (broadcast for free);
    1/d via reciprocal_approx_fast; ctx normalized by one STT per bank
  - ctx^T packed 4 heads per 128-partition tile -> full-K out-projection
  - inputs/weights packed into a handful of wide [128, N] tiles; tail-only
    tensors (xqtok, ln params) are DMA'd late, gated on proj completion, so
    they don't contend with the critical loads for HBM fabric
  - v2 projections run INSIDE the attention phase, using the U/dT PSUM banks
    as transient scratch before their accumulations start (their start=True
    wipes the scratch), with the U/d matmul backlog drained once the scratch
    reads complete — this overlaps ~3us of projection with the exp stream
  - LN gain/bias multiplies are skipped when the runtime values are
    trivially 1/0 (program is specialized per call)
"""

import math
from contextlib import ExitStack

import numpy as np
import ml_dtypes

import concourse.bass as bass
import concourse.tile as tile
from concourse import bacc, mybir
from concourse.bass_utils import run_bass_kernel_spmd

QD, KD, HID, NH = 256, 320, 256, 8
NQ, NK, NB = 4096, 4096, 16
DH = HID // NH
EPS = 1e-5
SCALE = 1.0 / math.sqrt(DH)
NCORES = 8
GPC = NB // NCORES  # graphs per core
F32 = mybir.dt.float32
BF16 = mybir.dt.bfloat16
NPBF16 = ml_dtypes.bfloat16
MASK_NEG = -30000.0


def _ceil(a, b):
    return -(-a // b)


def _build_program(QBs, KBs, ln_trivial, KBC_REAL=None):
    KTs = [kb // 128 for kb in KBs]
    KTT = sum(KTs)
    NQC = sum(QBs)
    KBC = sum(KBs)
    QBM = max(QBs)
    qofs = [0, QBs[0]]
    kofs = [0, KBs[0]]
    ktofs = [0, KTs[0]]
    if KBC_REAL is None:
        KBC_REAL = KBC
    assert QBM <= 512 and max(KBs) <= 512
    QT = _ceil(NQC, 128)  # token tiles for out-proj/LN

    nc = bacc.Bacc(
        "TRN2", target_bir_lowering=False, debug=False, num_devices=NCORES
    )
    # packed inputs (see kernel() for layouts)
    xq_d = nc.declare_dram_parameter("xq", [128, 2 * NQC], BF16, isOutput=False)
    xk_d = nc.declare_dram_parameter("xk", [128, 3 * KBC], BF16, isOutput=False)
    w_d = nc.declare_dram_parameter("w", [128, 2560], BF16, isOutput=False)
    xqtok_d = nc.declare_dram_parameter("xqtok", [128, QT * QD], F32, isOutput=False)
    sm_d = nc.declare_dram_parameter("sm", [128, 4 + KTT], F32, isOutput=False)
    lng_d = nc.declare_dram_parameter("lng", [QD], F32, isOutput=False)
    lnb_d = nc.declare_dram_parameter("lnb", [QD], F32, isOutput=False)
    out_d = nc.declare_dram_parameter("out", [NQC, QD], BF16, isOutput=True)

    WQ, WK, WV, WO = 0, 512, 1280, 2048  # col offsets in w_d

    with tile.TileContext(nc) as tc, ExitStack() as ctx:
        P = ctx.enter_context(tc.tile_pool(name="persist", bufs=1))

        # ---- loads (sm first: its bias/mask cols gate the proj bias-adds;
        # w split across two queues so wq/wk land early) ----
        sm = P.tile([128, 4 + KTT], F32, tag="sm", name="sm")
        nc.scalar.dma_start(out=sm, in_=sm_d[:, :])
        w = P.tile([128, 2560], BF16, tag="w", name="w")
        nc.scalar.dma_start(out=w[:, 0:512], in_=w_d[:, 0:512])
        xq = P.tile([128, 2 * NQC], BF16, tag="xq", name="xq")
        for kc in range(2):
            nc.sync.dma_start(
                out=xq[:, kc * NQC : (kc + 1) * NQC],
                in_=xq_d[:, kc * NQC : (kc + 1) * NQC],
            )
        # xk: zero the pad regions with memsets and only transfer the live
        # bytes (cols < KBC_REAL; block 2 holds KD-256=64 feature rows)
        xk = P.tile([128, 3 * KBC], BF16, tag="xk", name="xk")
        if KBC_REAL < KBC:
            for kc in range(3):
                nc.gpsimd.memset(
                    xk[:, kc * KBC + KBC_REAL : (kc + 1) * KBC], 0.0
                )
        nc.gpsimd.memset(xk[64:128, 2 * KBC : 2 * KBC + KBC_REAL], 0.0)
        for kc in range(2):
            nc.gpsimd.dma_start(
                out=xk[:, kc * KBC : kc * KBC + KBC_REAL],
                in_=xk_d[:, kc * KBC : kc * KBC + KBC_REAL],
            )
        nc.gpsimd.dma_start(
            out=xk[0:64, 2 * KBC : 2 * KBC + KBC_REAL],
            in_=xk_d[0:64, 2 * KBC : 2 * KBC + KBC_REAL],
        )
        nc.scalar.dma_start(out=w[:, 512:1280], in_=w_d[:, 512:1280])
        nc.sync.dma_start(out=w[:, 1280:2560], in_=w_d[:, 1280:2560])
        # tail-only tensors: tiles declared here, DMAs issued after the proj
        # phase (gated on proj outputs) so they don't contend for HBM fabric
        xqtok = P.tile([128, QT * QD], F32, tag="xqtok", name="xqtok")
        lng = P.tile([128, QD], F32, tag="lng", name="lng")
        lnb = P.tile([128, QD], F32, tag="lnb", name="lnb")
        ones = P.tile([128, 32], BF16, tag="ones", name="ones")
        nc.gpsimd.memset(ones, 1.0)
        epst = P.tile([128, 1], F32, tag="epst", name="epst")
        nc.gpsimd.memset(epst, EPS)

        q2T = [P.tile([128, NQC], BF16, tag=f"q2T{t}", name=f"q2T{t}") for t in range(2)]
        k2T = [P.tile([128, KBC], BF16, tag=f"k2T{t}", name=f"k2T{t}") for t in range(2)]
        v2 = [P.tile([128, HID], BF16, tag=f"v2_{i}", name=f"v2_{i}") for i in range(KTT)]
        ctxT = [P.tile([128, NQC], BF16, tag=f"ctxT{b}", name=f"ctxT{b}") for b in range(2)]
        rcp = P.tile([128, 2, QBM], F32, tag="rcp", name="rcp")

        def nsplits(total):
            return [(a, min(a + 512, total)) for a in range(0, total, 512)]

        # ---- projections ----
        with tc.tile_pool(name="proj_ps", bufs=2, space="PSUM") as pp:
            for t in range(2):
                ps = pp.tile([128, 2, 512], F32, tag="qk_ps", name="qk_ps")
                for kc in range(2):
                    for ci, (a, b) in enumerate(nsplits(NQC)):
                        nc.tensor.matmul(
                            ps[:, ci, 0 : b - a],
                            lhsT=w[:, WQ + 256 * kc + 128 * t : WQ + 256 * kc + 128 * t + 128],
                            rhs=xq[:, kc * NQC + a : kc * NQC + b],
                            start=(kc == 0),
                            stop=(kc == 1),
                        )
                for ci, (a, b) in enumerate(nsplits(NQC)):
                    nc.vector.tensor_scalar(
                        out=q2T[t][:, a:b],
                        in0=ps[:, ci, 0 : b - a],
                        scalar1=sm[:, t : t + 1],
                        scalar2=None,
                        op0=mybir.AluOpType.add,
                    )
            for t in range(2):
                if KBC_REAL < KBC:
                    nc.gpsimd.memset(k2T[t][:, KBC_REAL:KBC], 0.0)
                ps = pp.tile([128, 2, 512], F32, tag="qk_ps", name="qk_ps")
                for kc in range(3):
                    for ci, (a, b) in enumerate(nsplits(KBC_REAL)):
                        nc.tensor.matmul(
                            ps[:, ci, 0 : b - a],
                            lhsT=w[:, WK + 256 * kc + 128 * t : WK + 256 * kc + 128 * t + 128],
                            rhs=xk[:, kc * KBC + a : kc * KBC + b],
                            start=(kc == 0),
                            stop=(kc == 2),
                        )
                for ci, (a, b) in enumerate(nsplits(KBC_REAL)):
                    nc.vector.tensor_scalar(
                        out=k2T[t][:, a:b],
                        in0=ps[:, ci, 0 : b - a],
                        scalar1=sm[:, 2 + t : 3 + t],
                        scalar2=None,
                        op0=mybir.AluOpType.add,
                    )

        # late loads: gate each on a tiny DVE copy that depends on the last
        # proj output, so these DMAs only enter the ring after proj
        for t_ in (xqtok, lng, lnb):
            nc.vector.tensor_copy(out=t_[0:1, 0:1], in_=k2T[1][0:1, 0:1])
        nc.gpsimd.dma_start(out=xqtok, in_=xqtok_d[:, :])
        nc.gpsimd.dma_start(
            out=lng,
            in_=bass.AP(tensor=lng_d.ap().tensor, offset=0, ap=[[0, 128], [1, QD]]),
        )
        nc.gpsimd.dma_start(
            out=lnb,
            in_=bass.AP(tensor=lnb_d.ap().tensor, offset=0, ap=[[0, 128], [1, QD]]),
        )

        # ---- attention ----
        # Heads processed in PAIRS with two 2-bank S tiles ping-ponging so the
        # scalar engine's exp activates run back-to-back: while exp(p) reads
        # S tile p%2, the PE writes S(p+1) into the other tile and drains
        # U/d(p-1) from E.
        Etiles = [P.tile([128, 2, QBM], BF16, tag=f"E{p}", name=f"E{p}") for p in range(8)]
        with (
            tc.tile_pool(name="s_ps", bufs=1, space="PSUM") as sp,
            tc.tile_pool(name="u_ps", bufs=1, space="PSUM") as up,
            tc.tile_pool(name="d_ps", bufs=1, space="PSUM") as dp,
        ):
            Sab = [
                sp.tile([128, 2, 512], F32, tag="Sa", name="Sa"),
                sp.tile([128, 2, 512], F32, tag="Sb", name="Sb"),
            ]
            U = up.tile([128, 2, 512], F32, tag="U", name="U")
            dT = dp.tile([128, 2, 512], F32, tag="dT", name="dT")

            def emit_ud(g, kt, pr):
                qb, KT = QBs[g], KTs[g]
                Eh = Etiles[4 * (kt % 2) + pr]
                for j in range(2):
                    h = 2 * pr + j
                    ph, hh = h // 4, h % 4
                    nc.tensor.matmul(
                        U[32 * hh : 32 * hh + 32, ph, 0:qb],
                        lhsT=v2[ktofs[g] + kt][:, 32 * h : 32 * h + 32],
                        rhs=Eh[:, j, 0:qb],
                        start=(kt == 0),
                        stop=(kt == KT - 1),
                        tile_position=(0, 32 * hh),
                        skip_group_check=True,
                    )
                for j in range(2):
                    h = 2 * pr + j
                    ph, hh = h // 4, h % 4
                    # M=32 all-ones lhsT: every output row is the softmax
                    # denominator -> the 1/d broadcast is free
                    nc.tensor.matmul(
                        dT[32 * hh : 32 * hh + 32, ph, 0:qb],
                        lhsT=ones[:, 0:32],
                        rhs=Eh[:, j, 0:qb],
                        start=(kt == 0),
                        stop=(kt == KT - 1),
                        tile_position=(0, 32 * hh),
                        skip_group_check=True,
                    )

            def emit_norm(g):
                # normalization: dT rows already hold d broadcast per head;
                # rcp = 1/d then ctxT = U * rcp. Per-bank chains (b0 fully
                # before b1) so the next graph's first U/d matmuls - which
                # WAR-wait on bank 0's readers - unblock ~1us sooner.
                qb = QBs[g]
                for b in range(2):
                    nc.vector.reciprocal_approx_fast(
                        out=rcp[:, b, 0:qb], in_=dT[:, b, 0:qb]
                    )
                    nc.vector.scalar_tensor_tensor(
                        out=ctxT[b][:, qofs[g] : qofs[g] + qb],
                        in0=U[:, b, 0:qb],
                        scalar=0.0,
                        in1=rcp[:, b, 0:qb],
                        op0=mybir.AluOpType.bypass,
                        op1=mybir.AluOpType.mult,
                    )

            def emit_v2(i):
                # v2 projection for tile i, using U/dT banks as transient
                # PSUM scratch (their accumulations start only after the
                # backlog drain below; start=True wipes the scratch)
                g2, kt2 = (0, i) if i < KTs[0] else (1, i - KTs[0])
                kb0 = kofs[g2] + 128 * kt2
                slot = [dT[:, 0, 0:HID], dT[:, 1, 0:HID],
                        U[:, 0, 0:HID], U[:, 1, 0:HID]][i % 4]
                for kc in range(3):
                    nc.tensor.matmul(
                        slot,
                        lhsT=xk[:, kc * KBC + kb0 : kc * KBC + kb0 + 128],
                        rhs=w[:, WV + 256 * kc : WV + 256 * kc + 256],
                        start=(kc == 0),
                        stop=(kc == 2),
                        skip_group_check=True,
                    )
                nc.vector.tensor_copy(out=v2[i], in_=slot)

            # flat software-pipelined pair stream: S(i), exp(i), with v2
            # projections interleaved into the first pairs and the U/d
            # matmuls drained once the v2 scratch banks are free
            pairs = [
                (g, kt, pr)
                for g in range(GPC)
                for kt in range(KTs[g])
                for pr in range(4)
            ]
            def emit_S(i):
                g, kt, pr = pairs[i]
                qb = QBs[g]
                Sp = Sab[i % 2]
                for j in range(2):
                    h = 2 * pr + j
                    t, r = h // 4, 32 * (h % 4)
                    nc.tensor.matmul(
                        Sp[:, j, 0:qb],
                        lhsT=k2T[t][
                            r : r + 32,
                            kofs[g] + 128 * kt : kofs[g] + 128 * kt + 128,
                        ],
                        rhs=q2T[t][r : r + 32, qofs[g] : qofs[g] + qb],
                        start=True,
                        stop=True,
                        tile_position=(r, 0),
                    )

            ud_done = 0
            emit_S(0)
            for i, (g, kt, pr) in enumerate(pairs):
                qb = QBs[g]
                nc.scalar.activation(
                    out=Etiles[4 * (kt % 2) + pr][:, :, 0:qb],
                    in_=Sab[i % 2][:, 0:2, 0:qb],
                    func=mybir.ActivationFunctionType.Exp,
                    bias=sm[:, 4 + ktofs[g] + kt : 5 + ktofs[g] + kt],
                )
                # next pair's scores go into the PE queue BEFORE the v2/UD
                # bursts so the exp stream never waits on them
                if i + 1 < len(pairs):
                    emit_S(i + 1)
                if i < KTT:
                    emit_v2(i)
                if i >= KTT:
                    while ud_done < i:
                        pg, pk, pp_ = pairs[ud_done]
                        emit_ud(pg, pk, pp_)
                        if pp_ == 3 and pk == KTs[pg] - 1:
                            emit_norm(pg)
                        ud_done += 1
            # prewarm the Sqrt/Identity act-table set now (off the LN
            # critical path) - the scalar engine is done with exps
            nc.scalar.activation(
                out=rcp[0:1, 0, 0:1],
                in_=epst[0:1, 0:1],
                func=mybir.ActivationFunctionType.Sqrt,
                bias=epst[0:1, 0:1],
            )
            while ud_done < len(pairs):
                pg, pk, pp_ = pairs[ud_done]
                emit_ud(pg, pk, pp_)
                if pp_ == 3 and pk == KTs[pg] - 1:
                    emit_norm(pg)
                ud_done += 1

        # ---- out-projection + residual + layernorm ----
        # all out-proj matmuls issue back-to-back into their own PSUM banks
        # (8 free after the attention pools close), then the LN chains stream
        # behind them on vector/scalar/gpsimd
        with (
            tc.tile_pool(name="o_ps", bufs=1, space="PSUM") as op,
            tc.tile_pool(name="ln_sb", bufs=5) as lp,
        ):
            pss = []
            for qt in range(QT):
                sz = min(128, NQC - 128 * qt)
                # full-bank tile so each qt's PSUM is bank-isolated (PE write
                # vs DVE read of another qt's tile in the same bank is fatal)
                ps = op.tile([128, 512], F32, tag=f"o_ps{qt}", name=f"o_ps{qt}")
                pss.append(ps)
                for b in range(2):
                    nc.tensor.matmul(
                        ps[0:sz, 0:QD],
                        lhsT=ctxT[b][:, 128 * qt : 128 * qt + sz],
                        rhs=w[:, WO + 256 * b : WO + 256 * b + 256],
                        start=(b == 0),
                        stop=(b == 1),
                    )
            for qt in range(QT):
                sz = min(128, NQC - 128 * qt)
                ps = pss[qt]
                x = lp.tile([128, QD], F32, tag="x", name="x")
                nc.vector.tensor_add(
                    x[0:sz, :], ps[0:sz, 0:QD], xqtok[0:sz, QD * qt : QD * qt + QD]
                )
                stats = lp.tile([128, 6], F32, tag="stats", name="stats")
                nc.vector.bn_stats(out=stats[0:sz, :], in_=x[0:sz, :])
                mv = lp.tile([128, 2], F32, tag="mv", name="mv")
                nc.vector.bn_aggr(out=mv[0:sz, :], in_=stats[0:sz, :])
                sd = lp.tile([128, 1], F32, tag="sd", name="sd")
                nc.scalar.activation(
                    out=sd[0:sz, :],
                    in_=mv[0:sz, 1:2],
                    func=mybir.ActivationFunctionType.Sqrt,
                    bias=epst[0:sz, 0:1],
                )
                rstd = lp.tile([128, 1], F32, tag="rstd", name="rstd")
                nc.vector.reciprocal_approx_fast(out=rstd[0:sz, :], in_=sd[0:sz, :])
                # z = (x - mu) * rstd in one DVE op (two per-partition scalars)
                z = lp.tile([128, QD], BF16, tag="z", name="z")
                nc.vector.tensor_scalar(
                    out=z[0:sz, :],
                    in0=x[0:sz, :],
                    scalar1=mv[0:sz, 0:1],
                    scalar2=rstd[0:sz, 0:1],
                    op0=mybir.AluOpType.subtract,
                    op1=mybir.AluOpType.mult,
                )
                if ln_trivial:
                    yb = z
                else:
                    y = lp.tile([128, QD], BF16, tag="y", name="y")
                    nc.gpsimd.tensor_mul(y[0:sz, :], z[0:sz, :], lng[0:sz, :])
                    yb = lp.tile([128, QD], BF16, tag="yb", name="yb")
                    nc.gpsimd.tensor_add(yb[0:sz, :], y[0:sz, :], lnb[0:sz, :])
                dma_eng = [nc.sync, nc.gpsimd, nc.scalar][qt % 3]
                dma_eng.dma_start(
                    out=out_d[128 * qt : 128 * qt + sz, :], in_=yb[0:sz, :]
                )

    nc.compile()
    return nc


def kernel(**inputs):
    xqf = np.ascontiguousarray(np.asarray(inputs["query_nodes"], dtype=np.float32))
    xkf = np.ascontiguousarray(np.asarray(inputs["key_nodes"], dtype=np.float32))
    qbi = np.asarray(inputs["query_batch_idx"]).astype(np.int64)
    kbi = np.asarray(inputs["key_batch_idx"]).astype(np.int64)
    Wq = np.asarray(inputs["Wq"], np.float32)
    Wk = np.asarray(inputs["Wk"], np.float32)
    Wv = np.asarray(inputs["Wv"], np.float32)
    bq0 = np.asarray(inputs["bq"], np.float32)
    bk0 = np.asarray(inputs["bk"], np.float32)
    bv0 = np.asarray(inputs["bv"], np.float32)
    W2 = np.asarray(inputs["in_proj_w"], np.float32)
    b2 = np.asarray(inputs["in_proj_b"], np.float32)
    mow = np.asarray(inputs["mha_ow"], np.float32)
    mob = np.asarray(inputs["mha_ob"], np.float32)
    Wo = np.asarray(inputs["Wo"], np.float32)
    bo = np.asarray(inputs["bo"], np.float32)
    lng = np.asarray(inputs["ln_g"], np.float32)
    lnb = np.asarray(inputs["ln_b"], np.float32)

    # host-side weight folding
    Wq_eff = (Wq @ W2[:HID].T) * SCALE
    bq_eff = (bq0 @ W2[:HID].T + b2[:HID]) * SCALE
    Wk_eff = Wk @ W2[HID : 2 * HID].T
    bk_eff = bk0 @ W2[HID : 2 * HID].T + b2[HID : 2 * HID]
    Wv_eff = Wv @ W2[2 * HID :].T
    bv_eff = bv0 @ W2[2 * HID :].T + b2[2 * HID :]
    Wout_eff = mow @ Wo
    bout = bv_eff @ Wout_eff + mob @ Wo + bo  # folded into residual

    qcnt = np.bincount(qbi, minlength=NB)
    kcnt = np.bincount(kbi, minlength=NB)
    qoff = np.concatenate([[0], np.cumsum(qcnt)])
    koff = np.concatenate([[0], np.cumsum(kcnt)])

    # slot assignment: biggest 8 graphs -> slot 0, rest -> slot 1; rank by
    # key count or query count, whichever minimizes the padded tile cost
    def _slots_for(order):
        return [order[:NCORES], order[NCORES:]]

    def _cost(sl):
        kts = sum(
            _ceil(max(int(kcnt[g]) for g in s), 128) for s in sl
        )
        qbs = sum(
            _ceil(max(int(qcnt[g]) for g in s), 8) * 8 for s in sl
        )
        return (kts, qbs)

    cands = [
        _slots_for(np.argsort(-kcnt, kind="stable")),
        _slots_for(np.argsort(-qcnt, kind="stable")),
    ]
    slot_graphs = min(cands, key=_cost)
    assign = [[int(slot_graphs[0][c]), int(slot_graphs[1][c])] for c in range(NCORES)]

    def pad8(v):
        return int(_ceil(max(int(v), 8), 8) * 8)

    def pad128(v):
        return int(_ceil(max(int(v), 1), 128) * 128)

    QBs = [pad8(max(qcnt[g] for g in slot_graphs[s])) for s in range(2)]
    KBs = [pad128(max(kcnt[g] for g in slot_graphs[s])) for s in range(2)]
    KTs = [kb // 128 for kb in KBs]
    KTT = sum(KTs)
    NQC = sum(QBs)
    KBC = sum(KBs)
    QT = _ceil(NQC, 128)
    qofs = [0, QBs[0]]
    kofs = [0, KBs[0]]
    ktofs = [0, KTs[0]]

    ln_trivial = bool(np.all(lng == 1.0) and np.all(lnb == 0.0))
    kreal1 = max(int(kcnt[g]) for g in slot_graphs[1])
    KBC_REAL = min(KBC, int(_ceil(kofs[1] + kreal1, 8) * 8))
    nc = _build_program(QBs, KBs, ln_trivial, KBC_REAL)

    # packed weight tile [128, 2560]: wq(2 blocks) wk(3) wv(3) wo(2), each
    # block = 128 input-feature rows x 256 output cols
    w_all = np.zeros((128, 2560), np.float32)
    for kc in range(2):
        w_all[:, 256 * kc : 256 * kc + 256] = Wq_eff[128 * kc : 128 * kc + 128]
    for kc in range(3):
        r0, r1 = 128 * kc, min(128 * kc + 128, KD)
        w_all[0 : r1 - r0, 512 + 256 * kc : 512 + 256 * kc + 256] = Wk_eff[r0:r1]
        w_all[0 : r1 - r0, 1280 + 256 * kc : 1280 + 256 * kc + 256] = Wv_eff[r0:r1]
    for b in range(2):
        w_all[:, 2048 + 256 * b : 2048 + 256 * b + 256] = Wout_eff[128 * b : 128 * b + 128]
    w_all = w_all.astype(NPBF16)

    in_maps = []
    for c in range(NCORES):
        xqT = np.zeros((256, NQC), np.float32)
        xkT = np.zeros((384, KBC), np.float32)
        xqtok = np.zeros((128, QT * QD), np.float32)
        sm = np.zeros((128, 4 + KTT), np.float32)
        sm[:, 0] = bq_eff[0:128]
        sm[:, 1] = bq_eff[128:256]
        sm[:, 2] = bk_eff[0:128]
        sm[:, 3] = bk_eff[128:256]
        for gi in range(GPC):
            g = assign[c][gi]
            nq = int(qcnt[g])
            nk = int(kcnt[g])
            qo, ko = qofs[gi], kofs[gi]
            if nq:
                rows = xqf[qoff[g] : qoff[g + 1]]
                xqT[:, qo : qo + nq] = rows.T
                resid = rows + bout
                toks = qo + np.arange(nq)
                xqtok[
                    (toks % 128)[:, None],
                    (toks // 128)[:, None] * QD + np.arange(QD)[None, :],
                ] = resid
            if nk:
                xkT[:KD, ko : ko + nk] = xkf[koff[g] : koff[g + 1]].T
            for kt in range(KTs[gi]):
                p = np.arange(128)
                sm[:, 4 + ktofs[gi] + kt] = np.where(128 * kt + p < nk, 0.0, MASK_NEG)
        xq_all = np.concatenate([xqT[0:128], xqT[128:256]], axis=1).astype(NPBF16)
        xk_all = np.concatenate(
            [xkT[0:128], xkT[128:256], xkT[256:384]], axis=1
        ).astype(NPBF16)
        in_maps.append(
            {
                "xq": xq_all,
                "xk": xk_all,
                "w": w_all.copy(),
                "xqtok": xqtok,
                "sm": sm,
                "lng": lng.copy(),
                "lnb": lnb.copy(),
            }
        )

    import os

    trace = bool(os.environ.get("BASS_TRACE"))
    tmpdir = os.environ.get("BASS_TRACE_DIR") or None
    if tmpdir:
        import shutil

        shutil.rmtree(tmpdir, ignore_errors=True)
        os.makedirs(tmpdir, exist_ok=True)
    res = run_bass_kernel_spmd(
        nc, in_maps, list(range(NCORES)), trace=trace, tmpdir=tmpdir
    )
    if getattr(res, "exec_time_ns", None):
        print(f"HW exec time: {res.exec_time_ns} ns")
    out = np.empty((NQ, QD), np.float32)
    for c in range(NCORES):
        oc = res.results[c]["out"]
        for gi in range(GPC):
            g = assign[c][gi]
            nq = int(qcnt[g])
            if nq:
                out[qoff[g] : qoff[g + 1]] = oc[qofs[gi] : qofs[gi] + nq].astype(
                    np.float32
                )
    return out

